# revision 37
# baseline (speedup 1.0000x reference)
"""Trainium2 Bass kernel for nn_Attention_8272107012450 (sparse_attention).

Strategy: data-parallel over batch (8 batches -> 8 NeuronCores). Each core
computes all 16 heads (12 global + 4 local) of its batch:

  S^T[k,q] = K @ Q^T          (TensorE, bf16, contraction dk=64)
  local heads: S^T *= gate^T  (gate = sigmoid(rel_w[k-q+n-1]/0.1); Toeplitz,
               expanded on the fly by sliding-window DMA reads of the
               2047-entry sigmoid table; the +/- index flip is folded into a
               reversed free-axis access pattern on the consuming DVE op)
  P^T = exp(scale * S^T)      (ScalarE; no max subtraction -- scores are O(5))
  [out^T ; denom] = [V | 1s-column] PV matmul (TensorE, ones column gives the
               softmax denominator for free in row DK of the PSUM result)
  p_attn^T = P^T / denom, out^T = out^T / denom  (VectorE + GpSimd)

All tensors ride in transposed layouts; the host gather step transposes back.
Compute in bf16 (rel err ~5e-3, tolerance 2e-2). DMA layouts are chosen so
every transfer has >=2KB contiguous runs (16KB for the big p_attn store).
"""
import numpy as np
import ml_dtypes

import bass_rust
import concourse.bass as bass
import concourse.bacc as bacc
import concourse.mybir as mybir
import concourse.tile as tile
from concourse.bass_utils import run_bass_kernel_spmd

F32 = mybir.dt.float32
F16 = mybir.dt.float16
BF16 = mybir.dt.bfloat16
BF16_NP = ml_dtypes.bfloat16

# Route Exp and Ln to the one table set containing both, so the ACT engine
# never switches sets mid-kernel (each switch costs ~2.7us). Other sets keep
# their list position (set ids are positional into act_info.json).
_orig_get_tables = None


def _patched_tables(arch):
    t = _orig_get_tables(arch)
    combined = "natural_log_exp_and_others"
    if combined in t:
        exp_ln = {mybir.ActivationFunctionType.Exp,
                  mybir.ActivationFunctionType.Ln}
        for name in t:
            if name != combined:
                t[name] = t[name] - exp_ln
    return t


def _install_table_patch():
    global _orig_get_tables
    if _orig_get_tables is None:
        _orig_get_tables = bacc.get_activation_tables
        bacc.get_activation_tables = _patched_tables

B, H, N, DK = 8, 16, 1024, 64
GLOBAL_NUM, LOCAL_NUM = 12, 4
NT = N // 128          # 8 k/q tiles per head
VE = DK + 1            # V row width incl. ones column
SCALE = 1.0 / float(np.sqrt(DK))


def _rev_free(ap_full, width):
    """Copy of a [P, width] AP with the free axis reversed (step -1)."""
    c = ap_full.copy()
    steps = [list(x) for x in c.ap]
    assert steps[-1][0] > 0 and steps[-1][1] == width
    fs = steps[-1][0]
    steps[-1][0] = -fs
    c.ap = bass_rust.VecI64Pair(steps)
    c.offset = c.offset + (width - 1) * fs
    return c


def _gate_src(rs_in, row):
    """AP reading rs[row, t*128 + p + j] for t in [0,8), p in [0,128),
    j in [0,1024) -> [128, 8*1024] (partition p, free (t, j))."""
    c = rs_in[row].copy()
    c.ap = bass_rust.VecI64Pair([[1, 128], [128, NT], [1, N]])
    return c


def build():
    _install_table_patch()
    nc = bacc.Bacc(None)
    # d-major layouts so the one-time loads have huge contiguous runs
    q_in = nc.dram_tensor("q", [DK, H * N], BF16, kind="ExternalInput")
    k_in = nc.dram_tensor("k", [DK, H * N], BF16, kind="ExternalInput")
    v_in = nc.dram_tensor("v", [128, H * NT * VE], BF16, kind="ExternalInput")
    aq_in = nc.dram_tensor("aq", [DK, LOCAL_NUM * N], BF16, kind="ExternalInput")
    ak_in = nc.dram_tensor("ak", [DK, LOCAL_NUM * N], BF16, kind="ExternalInput")
    rs_in = nc.dram_tensor("rs", [LOCAL_NUM, 2 * N], BF16, kind="ExternalInput")

    # p_attn^T stored partition-major: [h, k%128, k//128, q] (16KB runs)
    pt_out = nc.dram_tensor("pt", [H, 128, NT * N], BF16, kind="ExternalOutput")
    ot_out = nc.dram_tensor("ot", [H, DK, N], F32, kind="ExternalOutput")

    EXP = mybir.ActivationFunctionType.Exp

    with (
        nc.allow_low_precision("bf16 attention kernel"),
        tile.TileContext(nc) as tc,
        tc.tile_pool(name="big", bufs=1) as big,
        tc.tile_pool(name="sb", bufs=2) as pool,
        tc.tile_pool(name="ptp", bufs=4) as ptp,
        tc.tile_pool(name="rbcp", bufs=4) as rbcp,
        tc.tile_pool(name="ps_s", bufs=3, space="PSUM") as psum_s,
        tc.tile_pool(name="ps_o", bufs=1, space="PSUM") as psum_o,
    ):
        # one-time loads; q/k duplicated onto partitions 64-127 so pairs of
        # K=64 S-matmuls can run concurrently in distinct PE row groups
        qt_all = big.tile([128, H * N], BF16)
        kt_all = big.tile([128, H * N], BF16)
        vo_all = big.tile([128, H * NT * VE], BF16)
        CH = 4 * N
        CHV = 4 * NT * VE
        for c in range(4):
            cs = slice(c * CH, (c + 1) * CH)
            nc.sync.dma_start(qt_all[0:DK, cs], q_in[:, cs])
            nc.sync.dma_start(qt_all[DK:128, cs], q_in[:, cs])
            nc.sync.dma_start(kt_all[0:DK, cs], k_in[:, cs])
            nc.sync.dma_start(kt_all[DK:128, cs], k_in[:, cs])
            nc.sync.dma_start(vo_all[:, c * CHV:(c + 1) * CHV],
                              v_in[:, c * CHV:(c + 1) * CHV])
        aq_s = pool.tile([128, LOCAL_NUM * N], BF16, tag="g")
        ak_s = pool.tile([128, LOCAL_NUM * N], BF16, tag="g")
        nc.sync.dma_start(aq_s[0:DK, :], aq_in[:])
        nc.sync.dma_start(aq_s[DK:128, :], aq_in[:])
        nc.sync.dma_start(ak_s[0:DK, :], ak_in[:])
        nc.sync.dma_start(ak_s[DK:128, :], ak_in[:])
        loc0 = GLOBAL_NUM * N
        nc.vector.tensor_add(qt_all[:, loc0:], qt_all[:, loc0:], aq_s[:])
        nc.vector.tensor_add(kt_all[:, loc0:], kt_all[:, loc0:], ak_s[:])

        def spv_phase(h):
            """S^T matmuls (2x row-packed) + gate + exp, with the PV matmuls
            interleaved two k-tile pairs behind so the PE has work while
            ScalarE runs exp."""
            is_local = h >= GLOBAL_NUM
            if is_local:
                g_all = pool.tile([128, NT * N], BF16, tag="g")
                nc.sync.dma_start(g_all[:], _gate_src(rs_in, h - GLOBAL_NUM))
            ptile = ptp.tile([128, NT * N], BF16, tag="ptile")
            po = psum_o.tile([VE, N], F32, tag="o")
            vo3 = vo_all[:, h * NT * VE:(h + 1) * NT * VE].rearrange(
                "p (t e) -> p t e", e=VE)

            def pv_pair(tp):
                for t in (2 * tp, 2 * tp + 1):
                    lhsT = vo3[:, t, :]
                    rhs = ptile[:, t * N:(t + 1) * N]
                    nc.tensor.matmul(po[:, 0:512], lhsT, rhs[:, 0:512],
                                     start=(t == 0), stop=(t == NT - 1))
                    nc.tensor.matmul(po[:, 512:1024], lhsT, rhs[:, 512:1024],
                                     start=(t == 0), stop=(t == NT - 1))

            lo = slice(0, DK)
            hi = slice(DK, 128)
            for tp in range(NT // 2):
                t0, t1 = 2 * tp, 2 * tp + 1
                ps0 = psum_s.tile([128, N], F32, tag="s")
                ps1 = psum_s.tile([128, N], F32, tag="s")
                kt0 = kt_all[lo, h * N + t0 * 128:h * N + (t0 + 1) * 128]
                kt1 = kt_all[hi, h * N + t1 * 128:h * N + (t1 + 1) * 128]
                q_lo = qt_all[lo, h * N:(h + 1) * N]
                q_hi = qt_all[hi, h * N:(h + 1) * N]
                nc.tensor.matmul(ps0[:, 0:512], kt0, q_lo[:, 0:512],
                                 start=True, stop=True)
                nc.tensor.matmul(ps0[:, 512:1024], kt0, q_lo[:, 512:1024],
                                 start=True, stop=True)
                nc.tensor.matmul(ps1[:, 0:512], kt1, q_hi[:, 0:512],
                                 start=True, stop=True)
                nc.tensor.matmul(ps1[:, 512:1024], kt1, q_hi[:, 512:1024],
                                 start=True, stop=True)
                if tp > 1:
                    pv_pair(tp - 2)
                for t, ps in ((t0, ps0), (t1, ps1)):
                    if is_local:
                        gseg = g_all[:, t * N:(t + 1) * N]
                        nc.vector.tensor_mul(ps[:], ps[:],
                                             _rev_free(gseg, N))
                    nc.scalar.activation(ptile[:, t * N:(t + 1) * N], ps[:],
                                         EXP, scale=SCALE)
            pv_pair(NT // 2 - 2)
            pv_pair(NT // 2 - 1)
            return ptile, po

        def norm_out(h, po, rbc):
            # frees the PSUM accumulator early so the next head's PV can run
            ot = pool.tile([DK, N], F32, tag="ot")
            nc.vector.tensor_mul(ot[:], po[:DK, :], rbc[:DK, :])
            nc.sync.dma_start(ot_out[h], ot[:])

        def norm_pt(h, ptile, rbc):
            cut = 7 * N
            nc.vector.tensor_mul(ptile[:, :cut], ptile[:, :cut],
                                 _bcast_t(rbc, 7))
            nc.gpsimd.tensor_mul(ptile[:, cut:], ptile[:, cut:], rbc[:])
            nc.sync.dma_start(pt_out[h], ptile[:])

        def _bcast_t(rbc, reps):
            """rbc [128, N] viewed as [128, reps, N] with 0-step over reps."""
            c = rbc[:].copy()
            steps = [list(x) for x in c.ap]
            steps = [steps[0], [0, reps], steps[1]]
            c.ap = bass_rust.VecI64Pair(steps)
            return c

        LN = mybir.ActivationFunctionType.Ln

        def make_rbc(po):
            # evacuate the PV accumulator to SBUF right away (ACT is idle at
            # the slot boundary) so the next head's PV matmuls can claim the
            # PSUM banks; then 1/d = exp(-ln(d)) on ACT -- Ln and Exp share
            # one table set, so no table switches and no 8-cyc/elem divide
            po_sb = pool.tile([VE, N], F32, tag="po_sb")
            nc.scalar.copy(po_sb[:], po[:])
            lnd = pool.tile([1, N], F32, tag="po_sb")
            nc.scalar.activation(lnd[:], po_sb[DK:DK + 1, :], LN)
            rec = pool.tile([1, N], BF16, tag="rec")
            nc.scalar.activation(rec[:], lnd[:], EXP, scale=-1.0)
            rbc = rbcp.tile([128, N], BF16, tag="rbc")
            nc.gpsimd.partition_broadcast(rbc[:], rec[:])
            return rbc, po_sb

        prev = None
        for h in range(H):
            if prev is not None:
                rbc_p, po_sb = make_rbc(prev[2])
                norm_out(prev[0], po_sb, rbc_p)
            cur = spv_phase(h)
            if prev is not None:
                norm_pt(prev[0], prev[1], rbc_p)
            prev = (h, cur[0], cur[1])
        rbc_p, po_sb = make_rbc(prev[2])
        norm_out(prev[0], po_sb, rbc_p)
        norm_pt(prev[0], prev[1], rbc_p)

    nc.compile()
    return nc


_CACHE = {}


def _get_nc():
    if "nc" not in _CACHE:
        _CACHE["nc"] = build()
    return _CACHE["nc"]


def prep_inputs(query, key, value, abs_q_w, abs_k_w, rel_w):
    """Host-side shard prep: bf16 casts, layout transposes, sigmoid table."""
    # q/k -> [B, DK, H*N] (d-major)
    qT = np.ascontiguousarray(np.transpose(query, (0, 3, 1, 2))
                              ).astype(BF16_NP).reshape(B, DK, H * N)
    kT = np.ascontiguousarray(np.transpose(key, (0, 3, 1, 2))
                              ).astype(BF16_NP).reshape(B, DK, H * N)
    # v -> [B, 128, H, NT, VE] with ones column at index DK
    v = np.ones((B, 128, H, NT, VE), BF16_NP)
    v[..., :DK] = value.reshape(B, H, NT, 128, DK).transpose(0, 3, 1, 2, 4)
    v = v.reshape(B, 128, H * NT * VE)
    # abs_*_w: raw reshape [N, LOCAL*DK] -> [LOCAL, N, DK] (matches the
    # reference's contiguous .reshape(1, LOCAL, n, dk)), -> [DK, LOCAL*N]
    aqT = np.ascontiguousarray(
        abs_q_w.reshape(LOCAL_NUM, N, DK).transpose(2, 0, 1)
    ).astype(BF16_NP).reshape(DK, LOCAL_NUM * N)
    akT = np.ascontiguousarray(
        abs_k_w.reshape(LOCAL_NUM, N, DK).transpose(2, 0, 1)
    ).astype(BF16_NP).reshape(DK, LOCAL_NUM * N)
    # sigmoid(rel_w/0.1): [2N-1, LOCAL] -> [LOCAL, 2N] padded
    x = np.asarray(rel_w, np.float32) * 10.0
    sig = (1.0 / (1.0 + np.exp(-x))).astype(np.float32)
    rs = np.zeros((LOCAL_NUM, 2 * N), np.float32)
    rs[:, :2 * N - 1] = sig.T
    rs = rs.astype(BF16_NP)
    return qT, kT, v, aqT, akT, rs


def run(inputs, trace=False, **kw):
    qT, kT, v, aqT, akT, rs = prep_inputs(
        inputs["query"], inputs["key"], inputs["value"],
        inputs["abs_q_w"], inputs["abs_k_w"], inputs["rel_w"])
    nc = _get_nc()
    in_maps = [{"q": qT[b], "k": kT[b], "v": v[b], "aq": aqT, "ak": akT,
                "rs": rs} for b in range(B)]
    res = run_bass_kernel_spmd(nc, in_maps, core_ids=list(range(B)),
                               trace=trace, **kw)
    out = np.empty((B, H, N, DK), np.float32)
    p_attn = np.empty((B, H, N, N), np.float32)
    for b in range(B):
        r = res.results[b]
        out[b] = np.transpose(r["ot"], (0, 2, 1))
        # pt: [h, p, t, q] -> p_attn[h, q, t*128+p]
        pt = r["pt"].reshape(H, 128, NT, N).astype(np.float32)
        p_attn[b] = pt.transpose(0, 3, 2, 1).reshape(H, N, N)
    return out, p_attn, res


def kernel(query, key, value, abs_q_w, abs_k_w, rel_w, mask):
    """Full-input entry point. mask is all-ones by construction -- unused."""
    inputs = {"query": np.asarray(query), "key": np.asarray(key),
              "value": np.asarray(value), "abs_q_w": np.asarray(abs_q_w),
              "abs_k_w": np.asarray(abs_k_w), "rel_w": np.asarray(rel_w)}
    out, p_attn, _ = run(inputs)
    return out, p_attn[:, :GLOBAL_NUM], p_attn[:, GLOBAL_NUM:]


# revision 38
# speedup vs baseline: 1.1186x; 1.1186x over previous
"""Trainium2 Bass kernel for nn_Attention_8272107012450 (sparse_attention).

Strategy: data-parallel over batch (8 batches -> 8 NeuronCores). Each core
computes all 16 heads (12 global + 4 local) of its batch:

  S^T[k,q] = K @ Q^T          (TensorE, bf16, contraction dk=64)
  local heads: S^T *= gate^T  (gate = sigmoid(rel_w[k-q+n-1]/0.1); Toeplitz,
               expanded on the fly by sliding-window DMA reads of the
               2047-entry sigmoid table; the +/- index flip is folded into a
               reversed free-axis access pattern on the consuming DVE op)
  P^T = exp(scale * S^T)      (ScalarE; no max subtraction -- scores are O(5))
  [out^T ; denom] = [V | 1s-column] PV matmul (TensorE, ones column gives the
               softmax denominator for free in row DK of the PSUM result)
  p_attn^T = P^T / denom, out^T = out^T / denom  (VectorE + GpSimd)

All tensors ride in transposed layouts; the host gather step transposes back.
Compute in bf16 (rel err ~5e-3, tolerance 2e-2). DMA layouts are chosen so
every transfer has >=2KB contiguous runs (16KB for the big p_attn store).
"""
import numpy as np
import ml_dtypes

import bass_rust
import concourse.bass as bass
import concourse.bacc as bacc
import concourse.mybir as mybir
import concourse.tile as tile
from concourse.bass_utils import run_bass_kernel_spmd

F32 = mybir.dt.float32
F16 = mybir.dt.float16
BF16 = mybir.dt.bfloat16
BF16_NP = ml_dtypes.bfloat16

# Route Exp and Ln to the one table set containing both, so the ACT engine
# never switches sets mid-kernel (each switch costs ~2.7us). Other sets keep
# their list position (set ids are positional into act_info.json).
_orig_get_tables = None


def _patched_tables(arch):
    t = _orig_get_tables(arch)
    combined = "natural_log_exp_and_others"
    if combined in t:
        exp_ln = {mybir.ActivationFunctionType.Exp,
                  mybir.ActivationFunctionType.Ln}
        for name in t:
            if name != combined:
                t[name] = t[name] - exp_ln
    return t


def _install_table_patch():
    global _orig_get_tables
    if _orig_get_tables is None:
        _orig_get_tables = bacc.get_activation_tables
        bacc.get_activation_tables = _patched_tables

B, H, N, DK = 8, 16, 1024, 64
GLOBAL_NUM, LOCAL_NUM = 12, 4
NT = N // 128          # 8 k/q tiles per head
VE = DK + 1            # V row width incl. ones column
SCALE = 1.0 / float(np.sqrt(DK))


def _rev_free(ap_full, width):
    """Copy of a [P, width] AP with the free axis reversed (step -1)."""
    c = ap_full.copy()
    steps = [list(x) for x in c.ap]
    assert steps[-1][0] > 0 and steps[-1][1] == width
    fs = steps[-1][0]
    steps[-1][0] = -fs
    c.ap = bass_rust.VecI64Pair(steps)
    c.offset = c.offset + (width - 1) * fs
    return c


def _gate_src(rs_in, row):
    """AP reading rs[row, t*128 + p + j] for t in [0,8), p in [0,128),
    j in [0,1024) -> [128, 8*1024] (partition p, free (t, j))."""
    c = rs_in[row].copy()
    c.ap = bass_rust.VecI64Pair([[1, 128], [128, NT], [1, N]])
    return c


def build():
    _install_table_patch()
    nc = bacc.Bacc(None)
    # d-major layouts so the one-time loads have huge contiguous runs
    q_in = nc.dram_tensor("q", [DK, H * N], BF16, kind="ExternalInput")
    k_in = nc.dram_tensor("k", [DK, H * N], BF16, kind="ExternalInput")
    v_in = nc.dram_tensor("v", [128, H * NT * VE], BF16, kind="ExternalInput")
    aq_in = nc.dram_tensor("aq", [DK, LOCAL_NUM * N], BF16, kind="ExternalInput")
    ak_in = nc.dram_tensor("ak", [DK, LOCAL_NUM * N], BF16, kind="ExternalInput")
    rs_in = nc.dram_tensor("rs", [LOCAL_NUM, 2 * N], BF16, kind="ExternalInput")

    # p_attn^T stored partition-major: [h, k%128, k//128, q] (16KB runs)
    pt_out = nc.dram_tensor("pt", [H, 128, NT * N], BF16, kind="ExternalOutput")
    ot_out = nc.dram_tensor("ot", [H, DK, N], F32, kind="ExternalOutput")

    EXP = mybir.ActivationFunctionType.Exp

    with (
        nc.allow_low_precision("bf16 attention kernel"),
        tile.TileContext(nc) as tc,
        tc.tile_pool(name="big", bufs=1) as big,
        tc.tile_pool(name="sb", bufs=2) as pool,
        tc.tile_pool(name="ptp", bufs=3) as ptp,
        tc.tile_pool(name="rbcp", bufs=4) as rbcp,
        tc.tile_pool(name="ps_s", bufs=3, space="PSUM") as psum_s,
        tc.tile_pool(name="ps_o", bufs=1, space="PSUM") as psum_o,
    ):
        # one-time loads; q/k duplicated onto partitions 64-127 so pairs of
        # K=64 S-matmuls can run concurrently in distinct PE row groups
        qt_all = big.tile([128, H * N], BF16)
        kt_all = big.tile([128, H * N], BF16)
        vo_all = big.tile([128, H * NT * VE], BF16)
        CH = 4 * N
        CHV = 4 * NT * VE
        for c in range(4):
            cs = slice(c * CH, (c + 1) * CH)
            nc.sync.dma_start(qt_all[0:DK, cs], q_in[:, cs])
            nc.sync.dma_start(qt_all[DK:128, cs], q_in[:, cs])
            nc.sync.dma_start(kt_all[0:DK, cs], k_in[:, cs])
            nc.sync.dma_start(kt_all[DK:128, cs], k_in[:, cs])
            nc.sync.dma_start(vo_all[:, c * CHV:(c + 1) * CHV],
                              v_in[:, c * CHV:(c + 1) * CHV])
        aq_s = pool.tile([128, LOCAL_NUM * N], BF16, tag="g")
        ak_s = pool.tile([128, LOCAL_NUM * N], BF16, tag="g")
        nc.sync.dma_start(aq_s[0:DK, :], aq_in[:])
        nc.sync.dma_start(aq_s[DK:128, :], aq_in[:])
        nc.sync.dma_start(ak_s[0:DK, :], ak_in[:])
        nc.sync.dma_start(ak_s[DK:128, :], ak_in[:])
        loc0 = GLOBAL_NUM * N
        nc.vector.tensor_add(qt_all[:, loc0:], qt_all[:, loc0:], aq_s[:])
        nc.vector.tensor_add(kt_all[:, loc0:], kt_all[:, loc0:], ak_s[:])

        def spv_phase(h):
            """S^T matmuls (2x row-packed) + gate + exp, with the PV matmuls
            interleaved two k-tile pairs behind so the PE has work while
            ScalarE runs exp."""
            is_local = h >= GLOBAL_NUM
            if is_local:
                g_all = pool.tile([128, NT * N], BF16, tag="g")
                nc.sync.dma_start(g_all[:], _gate_src(rs_in, h - GLOBAL_NUM))
            ptile = ptp.tile([128, NT * N], BF16, tag="ptile")
            po = psum_o.tile([VE, N], F32, tag="o")
            vo3 = vo_all[:, h * NT * VE:(h + 1) * NT * VE].rearrange(
                "p (t e) -> p t e", e=VE)

            def pv_pair(tp):
                for t in (2 * tp, 2 * tp + 1):
                    lhsT = vo3[:, t, :]
                    rhs = ptile[:, t * N:(t + 1) * N]
                    nc.tensor.matmul(po[:, 0:512], lhsT, rhs[:, 0:512],
                                     start=(t == 0), stop=(t == NT - 1))
                    nc.tensor.matmul(po[:, 512:1024], lhsT, rhs[:, 512:1024],
                                     start=(t == 0), stop=(t == NT - 1))

            lo = slice(0, DK)
            hi = slice(DK, 128)
            for tp in range(NT // 2):
                t0, t1 = 2 * tp, 2 * tp + 1
                ps0 = psum_s.tile([128, N], F32, tag="s")
                ps1 = psum_s.tile([128, N], F32, tag="s")
                kt0 = kt_all[lo, h * N + t0 * 128:h * N + (t0 + 1) * 128]
                kt1 = kt_all[hi, h * N + t1 * 128:h * N + (t1 + 1) * 128]
                q_lo = qt_all[lo, h * N:(h + 1) * N]
                q_hi = qt_all[hi, h * N:(h + 1) * N]
                nc.tensor.matmul(ps0[:, 0:512], kt0, q_lo[:, 0:512],
                                 start=True, stop=True)
                nc.tensor.matmul(ps0[:, 512:1024], kt0, q_lo[:, 512:1024],
                                 start=True, stop=True)
                nc.tensor.matmul(ps1[:, 0:512], kt1, q_hi[:, 0:512],
                                 start=True, stop=True)
                nc.tensor.matmul(ps1[:, 512:1024], kt1, q_hi[:, 512:1024],
                                 start=True, stop=True)
                if tp > 1:
                    pv_pair(tp - 2)
                for t, ps in ((t0, ps0), (t1, ps1)):
                    if is_local:
                        gseg = g_all[:, t * N:(t + 1) * N]
                        nc.vector.tensor_mul(ps[:], ps[:],
                                             _rev_free(gseg, N))
                    nc.scalar.activation(ptile[:, t * N:(t + 1) * N], ps[:],
                                         EXP, scale=SCALE)
            pv_pair(NT // 2 - 2)
            pv_pair(NT // 2 - 1)
            return ptile, po

        def norm_out(h, po, rbc):
            # frees the PSUM accumulator early so the next head's PV can run
            ot = pool.tile([DK, N], F32, tag="ot")
            nc.vector.tensor_mul(ot[:], po[:DK, :], rbc[:DK, :])
            nc.sync.dma_start(ot_out[h], ot[:])

        def norm_pt(h, ptile, rbc):
            cut = 7 * N
            nc.vector.tensor_mul(ptile[:, :cut], ptile[:, :cut],
                                 _bcast_t(rbc, 7))
            nc.gpsimd.tensor_mul(ptile[:, cut:], ptile[:, cut:], rbc[:])
            nc.sync.dma_start(pt_out[h], ptile[:])

        def _bcast_t(rbc, reps):
            """rbc [128, N] viewed as [128, reps, N] with 0-step over reps."""
            c = rbc[:].copy()
            steps = [list(x) for x in c.ap]
            steps = [steps[0], [0, reps], steps[1]]
            c.ap = bass_rust.VecI64Pair(steps)
            return c

        LN = mybir.ActivationFunctionType.Ln

        def make_rbc(po):
            # evacuate the PV accumulator to SBUF right away (ACT is idle at
            # the slot boundary) so the next head's PV matmuls can claim the
            # PSUM banks; then 1/d = exp(-ln(d)) on ACT -- Ln and Exp share
            # one table set, so no table switches and no 8-cyc/elem divide
            po_sb = pool.tile([VE, N], F32, tag="po_sb")
            nc.scalar.copy(po_sb[:], po[:])
            lnd = pool.tile([1, N], F32, tag="lnd")
            nc.scalar.activation(lnd[:], po_sb[DK:DK + 1, :], LN)
            rec = pool.tile([1, N], BF16, tag="rec")
            nc.scalar.activation(rec[:], lnd[:], EXP, scale=-1.0)
            rbc = rbcp.tile([128, N], BF16, tag="rbc")
            nc.gpsimd.partition_broadcast(rbc[:], rec[:])
            return rbc, po_sb

        prev = None
        for h in range(H):
            if prev is not None:
                rbc_p, po_sb = make_rbc(prev[2])
                norm_out(prev[0], po_sb, rbc_p)
            cur = spv_phase(h)
            if prev is not None:
                norm_pt(prev[0], prev[1], rbc_p)
            prev = (h, cur[0], cur[1])
        rbc_p, po_sb = make_rbc(prev[2])
        norm_out(prev[0], po_sb, rbc_p)
        norm_pt(prev[0], prev[1], rbc_p)

    nc.compile()
    return nc


_CACHE = {}


def _get_nc():
    if "nc" not in _CACHE:
        _CACHE["nc"] = build()
    return _CACHE["nc"]


def prep_inputs(query, key, value, abs_q_w, abs_k_w, rel_w):
    """Host-side shard prep: bf16 casts, layout transposes, sigmoid table."""
    # q/k -> [B, DK, H*N] (d-major)
    qT = np.ascontiguousarray(np.transpose(query, (0, 3, 1, 2))
                              ).astype(BF16_NP).reshape(B, DK, H * N)
    kT = np.ascontiguousarray(np.transpose(key, (0, 3, 1, 2))
                              ).astype(BF16_NP).reshape(B, DK, H * N)
    # v -> [B, 128, H, NT, VE] with ones column at index DK
    v = np.ones((B, 128, H, NT, VE), BF16_NP)
    v[..., :DK] = value.reshape(B, H, NT, 128, DK).transpose(0, 3, 1, 2, 4)
    v = v.reshape(B, 128, H * NT * VE)
    # abs_*_w: raw reshape [N, LOCAL*DK] -> [LOCAL, N, DK] (matches the
    # reference's contiguous .reshape(1, LOCAL, n, dk)), -> [DK, LOCAL*N]
    aqT = np.ascontiguousarray(
        abs_q_w.reshape(LOCAL_NUM, N, DK).transpose(2, 0, 1)
    ).astype(BF16_NP).reshape(DK, LOCAL_NUM * N)
    akT = np.ascontiguousarray(
        abs_k_w.reshape(LOCAL_NUM, N, DK).transpose(2, 0, 1)
    ).astype(BF16_NP).reshape(DK, LOCAL_NUM * N)
    # sigmoid(rel_w/0.1): [2N-1, LOCAL] -> [LOCAL, 2N] padded
    x = np.asarray(rel_w, np.float32) * 10.0
    sig = (1.0 / (1.0 + np.exp(-x))).astype(np.float32)
    rs = np.zeros((LOCAL_NUM, 2 * N), np.float32)
    rs[:, :2 * N - 1] = sig.T
    rs = rs.astype(BF16_NP)
    return qT, kT, v, aqT, akT, rs


def run(inputs, trace=False, **kw):
    qT, kT, v, aqT, akT, rs = prep_inputs(
        inputs["query"], inputs["key"], inputs["value"],
        inputs["abs_q_w"], inputs["abs_k_w"], inputs["rel_w"])
    nc = _get_nc()
    in_maps = [{"q": qT[b], "k": kT[b], "v": v[b], "aq": aqT, "ak": akT,
                "rs": rs} for b in range(B)]
    res = run_bass_kernel_spmd(nc, in_maps, core_ids=list(range(B)),
                               trace=trace, **kw)
    out = np.empty((B, H, N, DK), np.float32)
    p_attn = np.empty((B, H, N, N), np.float32)
    for b in range(B):
        r = res.results[b]
        out[b] = np.transpose(r["ot"], (0, 2, 1))
        # pt: [h, p, t, q] -> p_attn[h, q, t*128+p]
        pt = r["pt"].reshape(H, 128, NT, N).astype(np.float32)
        p_attn[b] = pt.transpose(0, 3, 2, 1).reshape(H, N, N)
    return out, p_attn, res


def kernel(query, key, value, abs_q_w, abs_k_w, rel_w, mask):
    """Full-input entry point. mask is all-ones by construction -- unused."""
    inputs = {"query": np.asarray(query), "key": np.asarray(key),
              "value": np.asarray(value), "abs_q_w": np.asarray(abs_q_w),
              "abs_k_w": np.asarray(abs_k_w), "rel_w": np.asarray(rel_w)}
    out, p_attn, _ = run(inputs)
    return out, p_attn[:, :GLOBAL_NUM], p_attn[:, GLOBAL_NUM:]


# revision 39
# speedup vs baseline: 1.1498x; 1.0279x over previous
"""Trainium2 Bass kernel for nn_Attention_8272107012450 (sparse_attention).

Strategy: data-parallel over batch (8 batches -> 8 NeuronCores). Each core
computes all 16 heads (12 global + 4 local) of its batch:

  S^T[k,q] = K @ Q^T          (TensorE, bf16, contraction dk=64)
  local heads: S^T *= gate^T  (gate = sigmoid(rel_w[k-q+n-1]/0.1); Toeplitz,
               expanded on the fly by sliding-window DMA reads of the
               2047-entry sigmoid table; the +/- index flip is folded into a
               reversed free-axis access pattern on the consuming DVE op)
  P^T = exp(scale * S^T)      (ScalarE; no max subtraction -- scores are O(5))
  [out^T ; denom] = [V | 1s-column] PV matmul (TensorE, ones column gives the
               softmax denominator for free in row DK of the PSUM result)
  p_attn^T = P^T / denom, out^T = out^T / denom  (VectorE + GpSimd)

All tensors ride in transposed layouts; the host gather step transposes back.
Compute in bf16 (rel err ~5e-3, tolerance 2e-2). DMA layouts are chosen so
every transfer has >=2KB contiguous runs (16KB for the big p_attn store).
"""
import numpy as np
import ml_dtypes

import bass_rust
import concourse.bass as bass
import concourse.bacc as bacc
import concourse.mybir as mybir
import concourse.tile as tile
from concourse.bass_utils import run_bass_kernel_spmd

F32 = mybir.dt.float32
F16 = mybir.dt.float16
BF16 = mybir.dt.bfloat16
BF16_NP = ml_dtypes.bfloat16

# Route Exp and Ln to the one table set containing both, so the ACT engine
# never switches sets mid-kernel (each switch costs ~2.7us). Other sets keep
# their list position (set ids are positional into act_info.json).
_orig_get_tables = None


def _patched_tables(arch):
    t = _orig_get_tables(arch)
    combined = "natural_log_exp_and_others"
    if combined in t:
        exp_ln = {mybir.ActivationFunctionType.Exp,
                  mybir.ActivationFunctionType.Ln}
        for name in t:
            if name != combined:
                t[name] = t[name] - exp_ln
    return t


def _install_table_patch():
    global _orig_get_tables
    if _orig_get_tables is None:
        _orig_get_tables = bacc.get_activation_tables
        bacc.get_activation_tables = _patched_tables

B, H, N, DK = 8, 16, 1024, 64
GLOBAL_NUM, LOCAL_NUM = 12, 4
NT = N // 128          # 8 k/q tiles per head
VE = DK + 1            # V row width incl. ones column
SCALE = 1.0 / float(np.sqrt(DK))


def _rev_free(ap_full, width):
    """Copy of a [P, width] AP with the free axis reversed (step -1)."""
    c = ap_full.copy()
    steps = [list(x) for x in c.ap]
    assert steps[-1][0] > 0 and steps[-1][1] == width
    fs = steps[-1][0]
    steps[-1][0] = -fs
    c.ap = bass_rust.VecI64Pair(steps)
    c.offset = c.offset + (width - 1) * fs
    return c


def _gate_src(rs_in, row):
    """AP reading rs[row, t*128 + p + j] for t in [0,8), p in [0,128),
    j in [0,1024) -> [128, 8*1024] (partition p, free (t, j))."""
    c = rs_in[row].copy()
    c.ap = bass_rust.VecI64Pair([[1, 128], [128, NT], [1, N]])
    return c


def build():
    _install_table_patch()
    nc = bacc.Bacc(None)
    # d-major layouts so the one-time loads have huge contiguous runs
    q_in = nc.dram_tensor("q", [DK, H * N], BF16, kind="ExternalInput")
    k_in = nc.dram_tensor("k", [DK, H * N], BF16, kind="ExternalInput")
    v_in = nc.dram_tensor("v", [128, H * NT * VE], BF16, kind="ExternalInput")
    aq_in = nc.dram_tensor("aq", [DK, LOCAL_NUM * N], BF16, kind="ExternalInput")
    ak_in = nc.dram_tensor("ak", [DK, LOCAL_NUM * N], BF16, kind="ExternalInput")
    rs_in = nc.dram_tensor("rs", [LOCAL_NUM, 2 * N], BF16, kind="ExternalInput")

    # p_attn^T stored partition-major: [h, k%128, k//128, q] (16KB runs)
    pt_out = nc.dram_tensor("pt", [H, 128, NT * N], BF16, kind="ExternalOutput")
    ot_out = nc.dram_tensor("ot", [H, DK, N], F32, kind="ExternalOutput")

    EXP = mybir.ActivationFunctionType.Exp

    with (
        nc.allow_low_precision("bf16 attention kernel"),
        tile.TileContext(nc) as tc,
        tc.tile_pool(name="big", bufs=1) as big,
        tc.tile_pool(name="sb", bufs=2) as pool,
        tc.tile_pool(name="ptp", bufs=3) as ptp,
        tc.tile_pool(name="rbcp", bufs=4) as rbcp,
        tc.tile_pool(name="ps_s", bufs=3, space="PSUM") as psum_s,
        tc.tile_pool(name="ps_o", bufs=1, space="PSUM") as psum_o,
    ):
        # one-time loads; q/k duplicated onto partitions 64-127 so pairs of
        # K=64 S-matmuls can run concurrently in distinct PE row groups
        qt_all = big.tile([128, H * N], BF16)
        kt_all = big.tile([128, H * N], BF16)
        vo_all = big.tile([128, H * NT * VE], BF16)
        CH = 4 * N
        CHV = 4 * NT * VE
        for c in range(4):
            cs = slice(c * CH, (c + 1) * CH)
            nc.sync.dma_start(qt_all[0:DK, cs], q_in[:, cs])
            nc.sync.dma_start(qt_all[DK:128, cs], q_in[:, cs])
            nc.sync.dma_start(kt_all[0:DK, cs], k_in[:, cs])
            nc.sync.dma_start(kt_all[DK:128, cs], k_in[:, cs])
            nc.sync.dma_start(vo_all[:, c * CHV:(c + 1) * CHV],
                              v_in[:, c * CHV:(c + 1) * CHV])
        aq_s = pool.tile([128, LOCAL_NUM * N], BF16, tag="g")
        ak_s = pool.tile([128, LOCAL_NUM * N], BF16, tag="g")
        nc.sync.dma_start(aq_s[0:DK, :], aq_in[:])
        nc.sync.dma_start(aq_s[DK:128, :], aq_in[:])
        nc.sync.dma_start(ak_s[0:DK, :], ak_in[:])
        nc.sync.dma_start(ak_s[DK:128, :], ak_in[:])
        loc0 = GLOBAL_NUM * N
        nc.vector.tensor_add(qt_all[:, loc0:], qt_all[:, loc0:], aq_s[:])
        nc.vector.tensor_add(kt_all[:, loc0:], kt_all[:, loc0:], ak_s[:])

        def spv_phase(h, after_first_exps=None):
            """S^T matmuls (2x row-packed) + gate + exp, with the PV matmuls
            interleaved two k-tile pairs behind so the PE has work while
            ScalarE runs exp. after_first_exps() is invoked right after
            pair 0's exps so the previous head's reciprocal chain queues on
            ACT *behind* them (it otherwise delays pair 1's psum recycling
            by ~3-7us -- the dominant per-head PE stall)."""
            is_local = h >= GLOBAL_NUM
            if is_local:
                g_all = pool.tile([128, NT * N], BF16, tag="g")
                nc.sync.dma_start(g_all[:], _gate_src(rs_in, h - GLOBAL_NUM))
            ptile = ptp.tile([128, NT * N], BF16, tag="ptile")
            po = psum_o.tile([VE, N], F32, tag="o")
            vo3 = vo_all[:, h * NT * VE:(h + 1) * NT * VE].rearrange(
                "p (t e) -> p t e", e=VE)

            def pv_pair(tp):
                for t in (2 * tp, 2 * tp + 1):
                    lhsT = vo3[:, t, :]
                    rhs = ptile[:, t * N:(t + 1) * N]
                    nc.tensor.matmul(po[:, 0:512], lhsT, rhs[:, 0:512],
                                     start=(t == 0), stop=(t == NT - 1))
                    nc.tensor.matmul(po[:, 512:1024], lhsT, rhs[:, 512:1024],
                                     start=(t == 0), stop=(t == NT - 1))

            lo = slice(0, DK)
            hi = slice(DK, 128)
            for tp in range(NT // 2):
                t0, t1 = 2 * tp, 2 * tp + 1
                ps0 = psum_s.tile([128, N], F32, tag="s")
                ps1 = psum_s.tile([128, N], F32, tag="s")
                kt0 = kt_all[lo, h * N + t0 * 128:h * N + (t0 + 1) * 128]
                kt1 = kt_all[hi, h * N + t1 * 128:h * N + (t1 + 1) * 128]
                q_lo = qt_all[lo, h * N:(h + 1) * N]
                q_hi = qt_all[hi, h * N:(h + 1) * N]
                nc.tensor.matmul(ps0[:, 0:512], kt0, q_lo[:, 0:512],
                                 start=True, stop=True)
                nc.tensor.matmul(ps0[:, 512:1024], kt0, q_lo[:, 512:1024],
                                 start=True, stop=True)
                nc.tensor.matmul(ps1[:, 0:512], kt1, q_hi[:, 0:512],
                                 start=True, stop=True)
                nc.tensor.matmul(ps1[:, 512:1024], kt1, q_hi[:, 512:1024],
                                 start=True, stop=True)
                if tp > 1:
                    pv_pair(tp - 2)
                for t, ps in ((t0, ps0), (t1, ps1)):
                    if is_local:
                        gseg = g_all[:, t * N:(t + 1) * N]
                        nc.vector.tensor_mul(ps[:], ps[:],
                                             _rev_free(gseg, N))
                    nc.scalar.activation(ptile[:, t * N:(t + 1) * N], ps[:],
                                         EXP, scale=SCALE)
                if tp == 0 and after_first_exps is not None:
                    after_first_exps()
            pv_pair(NT // 2 - 2)
            pv_pair(NT // 2 - 1)
            return ptile, po

        def norm_out(h, po, rbc):
            # frees the PSUM accumulator early so the next head's PV can run
            ot = pool.tile([DK, N], F32, tag="ot")
            nc.vector.tensor_mul(ot[:], po[:DK, :], rbc[:DK, :])
            nc.sync.dma_start(ot_out[h], ot[:])

        def norm_pt(h, ptile, rbc):
            cut = 7 * N
            nc.vector.tensor_mul(ptile[:, :cut], ptile[:, :cut],
                                 _bcast_t(rbc, 7))
            nc.gpsimd.tensor_mul(ptile[:, cut:], ptile[:, cut:], rbc[:])
            nc.sync.dma_start(pt_out[h], ptile[:])

        def _bcast_t(rbc, reps):
            """rbc [128, N] viewed as [128, reps, N] with 0-step over reps."""
            c = rbc[:].copy()
            steps = [list(x) for x in c.ap]
            steps = [steps[0], [0, reps], steps[1]]
            c.ap = bass_rust.VecI64Pair(steps)
            return c

        LN = mybir.ActivationFunctionType.Ln

        def make_rbc(po):
            # evacuate the PV accumulator to SBUF right away (ACT is idle at
            # the slot boundary) so the next head's PV matmuls can claim the
            # PSUM banks; then 1/d = exp(-ln(d)) on ACT -- Ln and Exp share
            # one table set, so no table switches and no 8-cyc/elem divide
            po_sb = pool.tile([VE, N], F32, tag="po_sb")
            nc.scalar.copy(po_sb[:], po[:])
            lnd = pool.tile([1, N], F32, tag="lnd")
            nc.scalar.activation(lnd[:], po_sb[DK:DK + 1, :], LN)
            rec = pool.tile([1, N], BF16, tag="rec")
            nc.scalar.activation(rec[:], lnd[:], EXP, scale=-1.0)
            rbc = rbcp.tile([128, N], BF16, tag="rbc")
            nc.gpsimd.partition_broadcast(rbc[:], rec[:])
            return rbc, po_sb

        prev = None
        box = {}

        def finish_prev():
            rbc_p, po_sb = make_rbc(prev[2])
            norm_out(prev[0], po_sb, rbc_p)
            box["rbc"] = rbc_p

        for h in range(H):
            cur = spv_phase(h, after_first_exps=(finish_prev if prev else None))
            if prev is not None:
                norm_pt(prev[0], prev[1], box["rbc"])
            prev = (h, cur[0], cur[1])
        finish_prev()
        norm_pt(prev[0], prev[1], box["rbc"])

    nc.compile()
    return nc


_CACHE = {}


def _get_nc():
    if "nc" not in _CACHE:
        _CACHE["nc"] = build()
    return _CACHE["nc"]


def prep_inputs(query, key, value, abs_q_w, abs_k_w, rel_w):
    """Host-side shard prep: bf16 casts, layout transposes, sigmoid table."""
    # q/k -> [B, DK, H*N] (d-major)
    qT = np.ascontiguousarray(np.transpose(query, (0, 3, 1, 2))
                              ).astype(BF16_NP).reshape(B, DK, H * N)
    kT = np.ascontiguousarray(np.transpose(key, (0, 3, 1, 2))
                              ).astype(BF16_NP).reshape(B, DK, H * N)
    # v -> [B, 128, H, NT, VE] with ones column at index DK
    v = np.ones((B, 128, H, NT, VE), BF16_NP)
    v[..., :DK] = value.reshape(B, H, NT, 128, DK).transpose(0, 3, 1, 2, 4)
    v = v.reshape(B, 128, H * NT * VE)
    # abs_*_w: raw reshape [N, LOCAL*DK] -> [LOCAL, N, DK] (matches the
    # reference's contiguous .reshape(1, LOCAL, n, dk)), -> [DK, LOCAL*N]
    aqT = np.ascontiguousarray(
        abs_q_w.reshape(LOCAL_NUM, N, DK).transpose(2, 0, 1)
    ).astype(BF16_NP).reshape(DK, LOCAL_NUM * N)
    akT = np.ascontiguousarray(
        abs_k_w.reshape(LOCAL_NUM, N, DK).transpose(2, 0, 1)
    ).astype(BF16_NP).reshape(DK, LOCAL_NUM * N)
    # sigmoid(rel_w/0.1): [2N-1, LOCAL] -> [LOCAL, 2N] padded
    x = np.asarray(rel_w, np.float32) * 10.0
    sig = (1.0 / (1.0 + np.exp(-x))).astype(np.float32)
    rs = np.zeros((LOCAL_NUM, 2 * N), np.float32)
    rs[:, :2 * N - 1] = sig.T
    rs = rs.astype(BF16_NP)
    return qT, kT, v, aqT, akT, rs


def run(inputs, trace=False, **kw):
    qT, kT, v, aqT, akT, rs = prep_inputs(
        inputs["query"], inputs["key"], inputs["value"],
        inputs["abs_q_w"], inputs["abs_k_w"], inputs["rel_w"])
    nc = _get_nc()
    in_maps = [{"q": qT[b], "k": kT[b], "v": v[b], "aq": aqT, "ak": akT,
                "rs": rs} for b in range(B)]
    res = run_bass_kernel_spmd(nc, in_maps, core_ids=list(range(B)),
                               trace=trace, **kw)
    out = np.empty((B, H, N, DK), np.float32)
    p_attn = np.empty((B, H, N, N), np.float32)
    for b in range(B):
        r = res.results[b]
        out[b] = np.transpose(r["ot"], (0, 2, 1))
        # pt: [h, p, t, q] -> p_attn[h, q, t*128+p]
        pt = r["pt"].reshape(H, 128, NT, N).astype(np.float32)
        p_attn[b] = pt.transpose(0, 3, 2, 1).reshape(H, N, N)
    return out, p_attn, res


def kernel(query, key, value, abs_q_w, abs_k_w, rel_w, mask):
    """Full-input entry point. mask is all-ones by construction -- unused."""
    inputs = {"query": np.asarray(query), "key": np.asarray(key),
              "value": np.asarray(value), "abs_q_w": np.asarray(abs_q_w),
              "abs_k_w": np.asarray(abs_k_w), "rel_w": np.asarray(rel_w)}
    out, p_attn, _ = run(inputs)
    return out, p_attn[:, :GLOBAL_NUM], p_attn[:, GLOBAL_NUM:]
